# revision 14
# baseline (speedup 1.0000x reference)
"""Trainium2 Bass kernel for nn_EuclideanDistance (retrieval_knn).

out = quantize(x_pad) @ quantize(temp)
  where temp  = [weight; broadcast(bias, L rows)],  bias = colsum(weight^2)/L
        x_pad = [x, ones(B, L)]
        quantize(t) = round(t/s)*s,  s = max(max|t|/127, 1e-12)  (per tensor)

Strategy: shard the stored-vector axis N=16384 across 8 cores (2048 each),
replicate x. Per-tensor scales sx, sw are global scalars computed on host.

Numerics: round(t/s) are integers |k| <= 127, exact in bf16; the integer
matmul accumulates exactly in f32 PSUM (|sum| <= 544*127^2 < 2^24), so the
bf16 PE matmul reproduces the reference fp32 computation to ~1e-5.

The kernel computes out^T (N on partitions): lhsT = quantized weight chunks,
rhs = quantized x^T. In this orientation the contribution of the L ones
columns x the bias rows --- sum_l k1*kb[n] = L*k1*kb[n], constant across B ---
is a per-partition scalar, folded for free into the PSUM-evacuate op
(out = (psum + c) * sx*sw). That removes the ragged 5th K-chunk: K = 4x128.
"""

import sys

import numpy as np

try:
    import concourse.bacc as bacc  # noqa: F401
except ImportError:  # fresh interpreter without the repo on sys.path
    sys.path.insert(0, "/opt/trn_rl_repo")

import concourse.bacc as bacc
import concourse.mybir as mybir
import concourse.tile as tile
from concourse import bass_utils

B, D, N = 1024, 512, 16384
NCORES = 8
NS = N // NCORES          # 2048 stored vectors per core
L = 32                    # split_square_len
QMAX = np.float32(127.0)  # 2**(8-1) - 1
MAGIC = 12582912.0        # 1.5 * 2**23: float32 round-to-nearest-even trick
KC = D // 128             # 4 K-chunks
NC = NS // 128            # 16 output-partition chunks
BT = B // 512             # 2 rhs tiles

F32 = mybir.dt.float32
BF16 = mybir.dt.bfloat16

_NC_CACHE = None


def _body(nc, tc, xT, w, sc, cb, outT):
    from contextlib import ExitStack

    ID = mybir.ActivationFunctionType.Identity
    ADD = mybir.AluOpType.add
    MULT = mybir.AluOpType.mult

    with ExitStack() as ctx:
        cpool = ctx.enter_context(tc.tile_pool(name="const", bufs=1))
        qpool = ctx.enter_context(tc.tile_pool(name="qk", bufs=1))
        spool = ctx.enter_context(tc.tile_pool(name="stage", bufs=3))
        ppool = ctx.enter_context(tc.tile_pool(name="psum", bufs=8, space="PSUM"))
        opool = ctx.enter_context(tc.tile_pool(name="osb", bufs=4))

        scv = cpool.tile([128, 4], F32, name="scv")
        nc.sync.dma_start(scv, sc)
        inv_sx = scv[:, 0:1]
        inv_sw = scv[:, 1:2]
        sxsw = scv[:, 2:3]
        magic = scv[:, 3:4]
        cbv = cpool.tile([128, 2 * NC], F32, name="cbv")
        nc.sync.dma_start(cbv, cb)

        JB = 512                   # w column-block width
        NJB = NS // JB             # 4 blocks
        JPB = JB // 128            # 4 output chunks per block

        # ---- x + w block 0: loads and quantize fully interleaved so the
        #      first matmul's operands clear the ACT/DVE FIFOs early ----
        kxs = []
        kwb0 = []
        for k in range(KC):
            wf = spool.tile([128, JB], F32, name="wf", tag=f"wf{k}", bufs=4)
            nc.sync.dma_start(wf, w[k * 128:(k + 1) * 128, 0:JB])
            xf = spool.tile([128, B], F32, name="xf", tag="xf", bufs=4)
            nc.sync.dma_start(xf, xT[k * 128:(k + 1) * 128, :])

            wm = spool.tile([128, JB], F32, name="wm", tag=f"wm{k}", bufs=4)
            nc.scalar.activation(wm, wf, ID, bias=magic, scale=inv_sw)
            kw = spool.tile([128, JB], BF16, name=f"kw{k}", tag=f"kw{k}",
                            bufs=4)
            nc.gpsimd.tensor_scalar_add(kw, wm, -MAGIC)
            kwb0.append(kw)

            xm = spool.tile([128, B], F32, name="xm", tag="xm", bufs=4)
            nc.scalar.activation(xm, xf, ID, bias=magic, scale=inv_sx)
            kx = qpool.tile([128, B], BF16, name=f"kx{k}", tag=f"kx{k}")
            nc.gpsimd.tensor_scalar_add(kx, xm, -MAGIC)
            kxs.append(kx)

        # ---- remaining w blocks: all loads emitted BEFORE any out-DMA so
        #      the sync HWDGE ring (strict FIFO) never convoys loads
        #      behind stores ----
        kwblocks = [kwb0]
        for jb in range(1, NJB):
            kwb = []
            for k in range(KC):
                wf = spool.tile([128, JB], F32, name="wf", tag=f"wf{k}",
                                bufs=4)
                nc.sync.dma_start(
                    wf, w[k * 128:(k + 1) * 128, jb * JB:(jb + 1) * JB])
                wm = spool.tile([128, JB], F32, name="wm", tag=f"wm{k}",
                                bufs=4)
                nc.scalar.activation(wm, wf, ID, bias=magic, scale=inv_sw)
                kw = spool.tile([128, JB], BF16, name=f"kw{k}",
                                tag=f"kw{k}", bufs=4)
                nc.gpsimd.tensor_scalar_add(kw, wm, -MAGIC)
                kwb.append(kw)
            kwblocks.append(kwb)

        for jb in range(NJB):
            kwb = kwblocks[jb]
            for jj in range(JPB):
                j = jb * JPB + jj
                ps = [ppool.tile([128, 512], F32, name="ps", tag="ps")
                      for _ in range(BT)]
                for k in range(KC):
                    lhsT = kwb[k][:, jj * 128:(jj + 1) * 128]
                    for b in range(BT):
                        nc.tensor.matmul(
                            ps[b], lhsT, kxs[k][:, b * 512:(b + 1) * 512],
                            start=(k == 0), stop=(k == KC - 1))
                ob = opool.tile([128, B], F32, name="ob", tag="ob")
                for b in range(BT):
                    # (psum + c_int) * (sx*sw); DVE owns every PSUM evac so
                    # its FIFO order matches psum-group completion order
                    nc.vector.tensor_scalar(ob[:, b * 512:(b + 1) * 512],
                                            ps[b], cbv[:, j:j + 1],
                                            sxsw, ADD, MULT)
                nc.sync.dma_start(outT[j * 128:(j + 1) * 128, :], ob)


def _build():
    global _NC_CACHE
    if _NC_CACHE is not None:
        return _NC_CACHE
    nc = bacc.Bacc("TRN2", target_bir_lowering=False, debug=False,
                   enable_asserts=False, num_devices=1)
    xT = nc.dram_tensor("xT", [D, B], F32, kind="ExternalInput").ap()
    w = nc.dram_tensor("w", [D, NS], F32, kind="ExternalInput").ap()
    sc = nc.dram_tensor("sc", [128, 4], F32, kind="ExternalInput").ap()
    cb = nc.dram_tensor("cb", [128, 2 * NC], F32, kind="ExternalInput").ap()
    outT = nc.dram_tensor("outT", [NS, B], F32, kind="ExternalOutput").ap()
    with tile.TileContext(nc) as tc:
        _body(nc, tc, xT, w, sc, cb, outT)
    nc.compile()
    _NC_CACHE = nc
    return nc


def _prepare_inputs(x, weight, split_square_len):
    assert x.shape == (B, D) and weight.shape == (D, N)
    assert int(split_square_len) == L

    x = np.ascontiguousarray(x, dtype=np.float32)
    weight = np.ascontiguousarray(weight, dtype=np.float32)

    # bias = colsum(weight^2)/L in f32, matching the reference
    bias = (np.einsum("dn,dn->n", weight, weight, dtype=np.float32)
            / np.float32(L)).astype(np.float32)

    # global per-tensor scales (f32 arithmetic to match jax)
    max_x = np.float32(max(np.abs(x).max(), np.float32(1.0)))
    sx = np.maximum(max_x / QMAX, np.float32(1e-12))
    max_w = np.float32(max(np.abs(weight).max(), np.abs(bias).max()))
    sw = np.maximum(max_w / QMAX, np.float32(1e-12))

    x_T = np.ascontiguousarray(x.T)  # [D, B]

    sc = np.zeros((128, 4), dtype=np.float32)
    sc[:, 0] = np.float32(1.0) / sx
    sc[:, 1] = np.float32(1.0) / sw
    sc[:, 2] = sx * sw
    sc[:, 3] = np.float32(MAGIC)

    # ones/bias rank-1 term: c[n] = L * round(1/sx) * round(bias[n]/sw),
    # exact integers; divides (not reciprocal-mults) to match the reference.
    k1 = np.float32(np.round(np.float32(1.0) / sx))
    kb = np.round(bias / sw).astype(np.float32)
    c_int = (np.float32(L) * k1) * kb          # exact in f32 (< 2^24)
    c_scaled = c_int * (sx * sw)

    in_maps = []
    for c in range(NCORES):
        sl = slice(c * NS, (c + 1) * NS)
        cb = np.concatenate([
            c_int[sl].reshape(NC, 128).T,      # [128, NC], col j = chunk j
            c_scaled[sl].reshape(NC, 128).T,
        ], axis=1).astype(np.float32)
        cb = np.ascontiguousarray(cb)
        in_maps.append({
            "xT": x_T,
            "w": np.ascontiguousarray(weight[:, sl]),
            "sc": sc,
            "cb": cb,
        })
    return in_maps


def _run(in_maps, **kwargs):
    nc = _build()
    return bass_utils.run_bass_kernel_spmd(
        nc, in_maps, core_ids=list(range(NCORES)), **kwargs)


def kernel(x, weight, split_square_len):
    in_maps = _prepare_inputs(x, weight, split_square_len)
    res = _run(in_maps)
    outT = np.concatenate([res.results[c]["outT"] for c in range(NCORES)],
                          axis=0)          # [N, B]
    return outT.T                          # [B, N] view


# revision 15
# speedup vs baseline: 3.4174x; 3.4174x over previous
"""Trainium2 Bass kernel for nn_EuclideanDistance (retrieval_knn).

out = quantize(x_pad) @ quantize(temp)
  where temp  = [weight; broadcast(bias, L rows)],  bias = colsum(weight^2)/L
        x_pad = [x, ones(B, L)]
        quantize(t) = round(t/s)*s,  s = max(max|t|/127, 1e-12)  (per tensor)

Strategy: shard the stored-vector axis N=16384 across 8 cores (2048 each),
replicate x. Per-tensor scales sx, sw are global scalars computed on host.

Numerics: round(t/s) are integers |k| <= 127, exact in bf16; the integer
matmul accumulates exactly in f32 PSUM (|sum| <= 544*127^2 < 2^24), so the
bf16 PE matmul reproduces the reference fp32 computation to ~1e-5.

The kernel computes out^T (N on partitions): lhsT = quantized weight chunks,
rhs = quantized x^T. In this orientation the contribution of the L ones
columns x the bias rows --- sum_l k1*kb[n] = L*k1*kb[n], constant across B ---
is a per-partition scalar, folded for free into the PSUM-evacuate op
(out = (psum + c) * sx*sw). That removes the ragged 5th K-chunk: K = 4x128.
"""

import sys

import numpy as np

try:
    import concourse.bacc as bacc  # noqa: F401
except ImportError:  # fresh interpreter without the repo on sys.path
    sys.path.insert(0, "/opt/trn_rl_repo")

import concourse.bacc as bacc
import concourse.mybir as mybir
import concourse.tile as tile
from concourse import bass_utils

B, D, N = 1024, 512, 16384
NCORES = 8
NS = N // NCORES          # 2048 stored vectors per core
L = 32                    # split_square_len
QMAX = np.float32(127.0)  # 2**(8-1) - 1
MAGIC = 12582912.0        # 1.5 * 2**23: float32 round-to-nearest-even trick
KC = D // 128             # 4 K-chunks
NC = NS // 128            # 16 output-partition chunks
BT = B // 512             # 2 rhs tiles

F32 = mybir.dt.float32
BF16 = mybir.dt.bfloat16

_NC_CACHE = None


def _body(nc, tc, xT, w, sc, cb, outT):
    from contextlib import ExitStack

    ID = mybir.ActivationFunctionType.Identity
    ADD = mybir.AluOpType.add
    MULT = mybir.AluOpType.mult

    with ExitStack() as ctx:
        cpool = ctx.enter_context(tc.tile_pool(name="const", bufs=1))
        qpool = ctx.enter_context(tc.tile_pool(name="qk", bufs=1))
        spool = ctx.enter_context(tc.tile_pool(name="stage", bufs=3))
        ppool = ctx.enter_context(tc.tile_pool(name="psum", bufs=8, space="PSUM"))
        opool = ctx.enter_context(tc.tile_pool(name="osb", bufs=4))

        scv = cpool.tile([128, 4], F32, name="scv")
        nc.sync.dma_start(scv, sc)
        inv_sx = scv[:, 0:1]
        inv_sw = scv[:, 1:2]
        sxsw = scv[:, 2:3]
        magic = scv[:, 3:4]
        cbv = cpool.tile([128, 2 * NC], F32, name="cbv")
        nc.sync.dma_start(cbv, cb)

        JB = 512                   # w column-block width
        NJB = NS // JB             # 4 blocks
        JPB = JB // 128            # 4 output chunks per block

        # ---- all input DMAs upfront on the sync ring (strict FIFO): block 0
        #      + x interleaved first, then the remaining w blocks, so no
        #      store ever convoys ahead of a load ----
        wfs = {}
        xfs = []
        for k in range(KC):
            wf = spool.tile([128, JB], F32, name="wf", tag=f"wf{k}", bufs=4)
            nc.sync.dma_start(wf, w[k * 128:(k + 1) * 128, 0:JB])
            wfs[(0, k)] = wf
            xf = spool.tile([128, B], F32, name="xf", tag="xf", bufs=4)
            nc.sync.dma_start(xf, xT[k * 128:(k + 1) * 128, :])
            xfs.append(xf)
        for jb in range(1, NJB):
            for k in range(KC):
                wf = spool.tile([128, JB], F32, name="wf", tag=f"wf{k}",
                                bufs=4)
                nc.sync.dma_start(
                    wf, w[k * 128:(k + 1) * 128, jb * JB:(jb + 1) * JB])
                wfs[(jb, k)] = wf

        # ---- quantize x + w block 0 (interleaved for early PE start) ----
        kxs = []
        kwblocks = [[None] * KC for _ in range(NJB)]
        for k in range(KC):
            wm = spool.tile([128, JB], F32, name="wm", tag=f"wm{k}", bufs=4)
            nc.scalar.activation(wm, wfs[(0, k)], ID, bias=magic,
                                 scale=inv_sw)
            kw = spool.tile([128, JB], BF16, name=f"kw{k}", tag=f"kw{k}",
                            bufs=4)
            nc.vector.tensor_scalar_add(kw, wm, -MAGIC)
            kwblocks[0][k] = kw

            xm = spool.tile([128, B], F32, name="xm", tag="xm", bufs=4)
            nc.scalar.activation(xm, xfs[k], ID, bias=magic, scale=inv_sx)
            kx = qpool.tile([128, B], BF16, name=f"kx{k}", tag=f"kx{k}")
            nc.vector.tensor_scalar_add(kx, xm, -MAGIC)
            kxs.append(kx)

        # ---- per block: quantize the NEXT block first (its data is in
        #      flight), then compute this block; keeps each engine's FIFO
        #      in execution order ----
        for jb in range(NJB):
            if jb + 1 < NJB:
                for k in range(KC):
                    wm = spool.tile([128, JB], F32, name="wm", tag=f"wm{k}",
                                    bufs=4)
                    nc.scalar.activation(wm, wfs[(jb + 1, k)], ID,
                                         bias=magic, scale=inv_sw)
                    kw = spool.tile([128, JB], BF16, name=f"kw{k}",
                                    tag=f"kw{k}", bufs=4)
                    nc.vector.tensor_scalar_add(kw, wm, -MAGIC)
                    kwblocks[jb + 1][k] = kw

            kwb = kwblocks[jb]
            for jj in range(JPB):
                j = jb * JPB + jj
                ps = ppool.tile([128, B], F32, name="ps", tag="ps", bufs=4)
                for k in range(KC):
                    lhsT = kwb[k][:, jj * 128:(jj + 1) * 128]
                    for b in range(BT):
                        nc.tensor.matmul(
                            ps[:, b * 512:(b + 1) * 512], lhsT,
                            kxs[k][:, b * 512:(b + 1) * 512],
                            start=(k == 0), stop=(k == KC - 1))
                ob = opool.tile([128, B], F32, name="ob", tag="ob")
                if j % 2 == 0:
                    # (psum + c_int) * (sx*sw) on DVE
                    nc.vector.tensor_scalar(ob, ps, cbv[:, j:j + 1],
                                            sxsw, ADD, MULT)
                else:
                    # psum * (sx*sw) + c_scaled on ACT
                    nc.scalar.activation(ob, ps, ID,
                                         bias=cbv[:, NC + j:NC + j + 1],
                                         scale=sxsw)
                nc.sync.dma_start(outT[j * 128:(j + 1) * 128, :], ob)


def _build():
    global _NC_CACHE
    if _NC_CACHE is not None:
        return _NC_CACHE
    nc = bacc.Bacc("TRN2", target_bir_lowering=False, debug=False,
                   enable_asserts=False, num_devices=1)
    xT = nc.dram_tensor("xT", [D, B], F32, kind="ExternalInput").ap()
    w = nc.dram_tensor("w", [D, NS], F32, kind="ExternalInput").ap()
    sc = nc.dram_tensor("sc", [128, 4], F32, kind="ExternalInput").ap()
    cb = nc.dram_tensor("cb", [128, 2 * NC], F32, kind="ExternalInput").ap()
    outT = nc.dram_tensor("outT", [NS, B], F32, kind="ExternalOutput").ap()
    with tile.TileContext(nc) as tc:
        _body(nc, tc, xT, w, sc, cb, outT)
    nc.compile()
    _NC_CACHE = nc
    return nc


def _prepare_inputs(x, weight, split_square_len):
    assert x.shape == (B, D) and weight.shape == (D, N)
    assert int(split_square_len) == L

    x = np.ascontiguousarray(x, dtype=np.float32)
    weight = np.ascontiguousarray(weight, dtype=np.float32)

    # bias = colsum(weight^2)/L in f32, matching the reference
    bias = (np.einsum("dn,dn->n", weight, weight, dtype=np.float32)
            / np.float32(L)).astype(np.float32)

    # global per-tensor scales (f32 arithmetic to match jax)
    max_x = np.float32(max(np.abs(x).max(), np.float32(1.0)))
    sx = np.maximum(max_x / QMAX, np.float32(1e-12))
    max_w = np.float32(max(np.abs(weight).max(), np.abs(bias).max()))
    sw = np.maximum(max_w / QMAX, np.float32(1e-12))

    x_T = np.ascontiguousarray(x.T)  # [D, B]

    sc = np.zeros((128, 4), dtype=np.float32)
    sc[:, 0] = np.float32(1.0) / sx
    sc[:, 1] = np.float32(1.0) / sw
    sc[:, 2] = sx * sw
    sc[:, 3] = np.float32(MAGIC)

    # ones/bias rank-1 term: c[n] = L * round(1/sx) * round(bias[n]/sw),
    # exact integers; divides (not reciprocal-mults) to match the reference.
    k1 = np.float32(np.round(np.float32(1.0) / sx))
    kb = np.round(bias / sw).astype(np.float32)
    c_int = (np.float32(L) * k1) * kb          # exact in f32 (< 2^24)
    c_scaled = c_int * (sx * sw)

    in_maps = []
    for c in range(NCORES):
        sl = slice(c * NS, (c + 1) * NS)
        cb = np.concatenate([
            c_int[sl].reshape(NC, 128).T,      # [128, NC], col j = chunk j
            c_scaled[sl].reshape(NC, 128).T,
        ], axis=1).astype(np.float32)
        cb = np.ascontiguousarray(cb)
        in_maps.append({
            "xT": x_T,
            "w": np.ascontiguousarray(weight[:, sl]),
            "sc": sc,
            "cb": cb,
        })
    return in_maps


def _run(in_maps, **kwargs):
    nc = _build()
    return bass_utils.run_bass_kernel_spmd(
        nc, in_maps, core_ids=list(range(NCORES)), **kwargs)


def kernel(x, weight, split_square_len):
    in_maps = _prepare_inputs(x, weight, split_square_len)
    res = _run(in_maps)
    outT = np.concatenate([res.results[c]["outT"] for c in range(NCORES)],
                          axis=0)          # [N, B]
    return outT.T                          # [B, N] view
